# revision 1
# baseline (speedup 1.0000x reference)
"""Trainium2 Bass kernel for nn_Decoder2 (GRU decoder, Keras reset_after GRUCell).

Reference computation (per batch row b, scanned over t = 0..T-1):
    x_t   = [o_{t-1}, feat_t]                  # [1+F]
    mx    = x_t @ K + ib                       # [3H]
    mh    = h_{t-1} @ Wr + rb                  # [3H]
    z     = sigmoid(mx[:H]   + mh[:H])
    r     = sigmoid(mx[H:2H] + mh[H:2H])
    cand  = tanh(mx[2H:] + r * mh[2H:])
    h_t   = z * h_{t-1} + (1-z) * cand
    o_t   = h_t @ dw + db                      # scalar output per row

Shapes: B=8192, T=96, F=64, H=256.

Strategy: pure data parallel over batch (1024 rows/core on 8 cores), no
collectives.  On-chip layout is feature-major: [feature -> partitions,
batch -> free dim], so TensorE consumes the tiny weights as stationary
operands directly and biases ride along as a ones-row in x.

Critical-path trick: for t>=1 the o-feedback contribution to the z/r gates,
o_{t-1}*k0 = h_{t-1} @ (dw k0^T) + db*k0, is folded into the recurrent
weights host-side (Wr' = Wr + dw@k0^T on the z/r columns, db*k0 into the
bias row), so the z/r matmuls depend only on h_{t-1} and the per-step
latency chain shrinks.  Only the single xh pass reads the o row of x,
which has a full step of slack.  t=0 uses the unfolded weights plus an
explicit K=1 pass against the externally supplied init input.
"""

import os
import sys

for _p in ("/root/.axon_site/_ro/trn_rl_repo", "/opt/trn_rl_repo"):
    if os.path.isdir(_p) and _p not in sys.path:
        sys.path.insert(0, _p)

from contextlib import ExitStack  # noqa: E402

import numpy as np  # noqa: E402

import concourse.bacc as bacc  # noqa: E402
import concourse.tile as tile  # noqa: E402
from concourse import mybir  # noqa: E402
from concourse import bass_utils  # noqa: E402

Alu = mybir.AluOpType
Act = mybir.ActivationFunctionType

B, T, F, H = 8192, 96, 64, 256
G3 = 3 * H              # 768 gate width
NCORES = 8
BL = B // NCORES        # 1024 batch rows per core
# x tile partition layout (SBUF APs must start at partition 0/32/64/96):
#   rows 0..63  = feat_t^T
#   row  64     = ones (bias row; rows 65..95 also memset 1.0, weights 0)
#   row  96     = o_{t-1} (prev dense output)
XROWS = 97
OROW = 96
ONESROW = 64


def build_nc(
    t_steps: int = T,
    bl: int = BL,
    nt: int = 2,
    compute_dt: str = "bfloat16",
    fold: bool = True,
):
    """Build (and compile) the per-core Bass program.

    compute_dt: dtype for SBUF state/gates and matmul operands.
      "bfloat16" -> bf16 matmuls (fp32 PSUM accumulation)
      "float32"  -> fp32 storage, matmuls issued as float32r (fast fp32 mode)
    """
    n = bl // nt                     # batch-tile free size (<=512: one PSUM bank)
    assert n <= 512
    cdt = getattr(mybir.dt, compute_dt)
    f32 = mybir.dt.float32
    mm_cast = (lambda ap: ap.bitcast(mybir.dt.float32r)) if compute_dt == "float32" else (lambda ap: ap)
    nch = H // 128                   # 2 chunks of 128 features for h
    assert nch == 2

    nc = bacc.Bacc("TRN2", target_bir_lowering=False, debug=False)

    featT = nc.dram_tensor("featT", [t_steps, F, bl], cdt, kind="ExternalInput").ap()
    h0T = nc.dram_tensor("h0T", [128, nch, bl], cdt, kind="ExternalInput").ap()
    o0 = nc.dram_tensor("o0", [1, bl], cdt, kind="ExternalInput").ap()
    # kxw row OROW: zeros for z/r columns (o-feedback folded into wrw there),
    # kernel[0] for the h columns.  wrw z/r columns carry Wr + dw@k0^T.
    kxw = nc.dram_tensor("kxw", [XROWS, G3], cdt, kind="ExternalInput").ap()
    wrw = nc.dram_tensor("wrw", [128, nch, G3], cdt, kind="ExternalInput").ap()
    # unfolded Wr z/r columns + full k0 row, for the t=0 step where the
    # o-feedback is the externally supplied init input, not h@dw
    wrz0 = nc.dram_tensor("wrz0", [128, nch, 2 * H], cdt, kind="ExternalInput").ap()
    kxo = nc.dram_tensor("kxo", [1, G3], cdt, kind="ExternalInput").ap()
    dww = nc.dram_tensor("dww", [128, nch], cdt, kind="ExternalInput").ap()
    ident = nc.dram_tensor("ident", [128, 128], cdt, kind="ExternalInput").ap()
    rbh = nc.dram_tensor("rbh", [128, nch], f32, kind="ExternalInput").ap()
    outT = nc.dram_tensor("outT", [t_steps, bl], f32, kind="ExternalOutput").ap()

    with tile.TileContext(nc) as tc, ExitStack() as ctx:
        const = ctx.enter_context(tc.tile_pool(name="const", bufs=1))
        hpool = ctx.enter_context(tc.tile_pool(name="h", bufs=3))
        xpool = ctx.enter_context(tc.tile_pool(name="x", bufs=6))
        zpool = ctx.enter_context(tc.tile_pool(name="z", bufs=3))
        rpool = ctx.enter_context(tc.tile_pool(name="r", bufs=3))
        apool = ctx.enter_context(tc.tile_pool(name="a", bufs=3))
        bpool = ctx.enter_context(tc.tile_pool(name="b", bufs=3))
        rhpool = ctx.enter_context(tc.tile_pool(name="rh", bufs=3))
        cpool = ctx.enter_context(tc.tile_pool(name="cand", bufs=3))
        opool = ctx.enter_context(tc.tile_pool(name="osb", bufs=4))
        pz = ctx.enter_context(tc.tile_pool(name="pz", bufs=1, space="PSUM"))
        pr = ctx.enter_context(tc.tile_pool(name="pr", bufs=1, space="PSUM"))
        phh = ctx.enter_context(tc.tile_pool(name="phh", bufs=1, space="PSUM"))
        pxh = ctx.enter_context(tc.tile_pool(name="pxh", bufs=1, space="PSUM"))

        # --- constants ---
        kx_sb = const.tile([XROWS, G3], cdt)
        nc.sync.dma_start(out=kx_sb, in_=kxw)
        wr_sb = const.tile([128, nch, G3], cdt)
        nc.sync.dma_start(out=wr_sb, in_=wrw)
        wrz0_sb = const.tile([128, nch, 2 * H], cdt)
        nc.sync.dma_start(out=wrz0_sb, in_=wrz0)
        # k0 staged at partition OROW so the t=0 K=1 matmul's stationary
        # and moving operands share a row group
        kxo_sb = const.tile([XROWS, G3], cdt)
        nc.sync.dma_start(out=kxo_sb[OROW:OROW + 1, :], in_=kxo)
        dw_sb = const.tile([128, nch], cdt)
        nc.sync.dma_start(out=dw_sb, in_=dww)
        id_sb = const.tile([128, 128], cdt)
        nc.sync.dma_start(out=id_sb, in_=ident)
        rb_sb = const.tile([128, nch], f32)
        nc.sync.dma_start(out=rb_sb, in_=rbh)

        # --- initial state ---
        h_prev = hpool.tile([128, nch, bl], cdt)
        nc.sync.dma_start(out=h_prev, in_=h0T)
        xs = {}
        for j in range(nt):
            xj = xpool.tile([XROWS, n], cdt, tag="x")
            nc.sync.dma_start(out=xj[0:F, :], in_=featT[0, :, j * n:(j + 1) * n])
            nc.gpsimd.memset(xj[ONESROW:XROWS, :], 1.0)
            nc.sync.dma_start(out=xj[OROW:OROW + 1, :], in_=o0[0:1, j * n:(j + 1) * n])
            xs[(0, j)] = xj

        def emit_o_mm(t, j, h_t):
            """o(t,j) = h(t,j) @ dw matmuls into a psum slot shared with hh.
            Emitted inside block (t+1, j), where h(t,j) is long ready."""
            bs = slice(j * n, (j + 1) * n)
            op = phh.tile([1, n], f32, tag="phh")
            nc.tensor.matmul(op[0:1, :], mm_cast(dw_sb[:, 0:1]),
                             mm_cast(h_t[:, 0, bs]), start=True, stop=False)
            nc.tensor.matmul(op[0:1, :], mm_cast(dw_sb[:, 1:2]),
                             mm_cast(h_t[:, 1, bs]), start=False, stop=True)
            return op

        def emit_o_copy(t, j, op):
            """psum -> SBUF copy of o(t,j) + output DMA (ACT queue position
            is deliberately after sig_r)."""
            bs = slice(j * n, (j + 1) * n)
            o_sb = opool.tile([1, n], f32, tag="osb")
            nc.scalar.activation(o_sb, op, Act.Copy)
            nc.sync.dma_start(out=outT[t:t + 1, bs], in_=o_sb)
            return o_sb

        h_hist = {-1: h_prev}
        for t in range(t_steps):
            h_new = hpool.tile([128, nch, bl], cdt, tag="h")
            h_hist[t] = h_new
            h_prev = h_hist[t - 1]
            for j in range(nt):
                bs = slice(j * n, (j + 1) * n)
                x = xs[(t, j)]

                def zr_gate_mms(gp, gi):
                    for c in range(nch):
                        m = (gi + c) * 128
                        if t == 0:
                            w0 = wrz0_sb[:, 0, m:m + 128]
                            w1 = wrz0_sb[:, 1, m:m + 128]
                        else:
                            w0 = wr_sb[:, 0, m:m + 128]
                            w1 = wr_sb[:, 1, m:m + 128]
                        nc.tensor.matmul(gp[:, c, :], mm_cast(w0),
                                         mm_cast(h_prev[:, 0, bs]),
                                         start=True, stop=False)
                        nc.tensor.matmul(gp[:, c, :], mm_cast(w1),
                                         mm_cast(h_prev[:, 1, bs]),
                                         start=False, stop=False)
                        # feat+ones rows only: z/r never read the o row (folded)
                        xtop = OROW if fold else XROWS
                        nc.tensor.matmul(gp[:, c, :],
                                         mm_cast(kx_sb[0:xtop, m:m + 128]),
                                         mm_cast(x[0:xtop, :]),
                                         start=False, stop=t != 0 or not fold)
                        if t == 0 and fold:
                            # o-feedback at t=0 is the external init input
                            # (x row OROW); K=1 matmul on PE row-group 3
                            nc.tensor.matmul(
                                gp[:, c, :],
                                mm_cast(kxo_sb[OROW:OROW + 1, m:m + 128]),
                                mm_cast(x[OROW:OROW + 1, :]),
                                start=False, stop=True, tile_position=(96, 0))

                # -- r gate matmuls head the chain
                rp = pr.tile([128, nch, n], f32, tag="pr")
                zr_gate_mms(rp, 2)
                r_sb = rpool.tile([128, nch, n], cdt, tag="r")
                for c in range(nch):
                    nc.scalar.activation(r_sb[:, c, :], rp[:, c, :], Act.Sigmoid)

                # -- deferred output work from step t-1 (h ready long ago).
                # MUST be emitted before the xh matmuls below: program order
                # defines the dataflow, so the o -> x-row copy has to precede
                # its reader.  Positioned after the r matmuls so the o
                # matmul's psum slot (shared with hh) is already free.
                if t > 0:
                    op = emit_o_mm(t - 1, j, h_prev)
                    o_sb = emit_o_copy(t - 1, j, op)
                    nc.vector.tensor_copy(out=x[OROW:OROW + 1, :], in_=o_sb)

                # -- prefetch next step's x tile
                if t < t_steps - 1:
                    xj = xpool.tile([XROWS, n], cdt, tag="x")
                    nc.sync.dma_start(
                        out=xj[0:F, :], in_=featT[t + 1, :, j * n:(j + 1) * n])
                    nc.gpsimd.memset(xj[ONESROW:XROWS, :], 1.0)
                    xs[(t + 1, j)] = xj

                # -- remaining gate matmuls: z, hh, then 1-pass xh
                zp = pz.tile([128, nch, n], f32, tag="pz")
                zr_gate_mms(zp, 0)
                hhp = phh.tile([128, nch, n], f32, tag="phh")
                for c in range(nch):
                    m = (4 + c) * 128
                    nc.tensor.matmul(
                        hhp[:, c, :], mm_cast(wr_sb[:, 0, m:m + 128]),
                        mm_cast(h_prev[:, 0, bs]), start=True, stop=False)
                    nc.tensor.matmul(
                        hhp[:, c, :], mm_cast(wr_sb[:, 1, m:m + 128]),
                        mm_cast(h_prev[:, 1, bs]), start=False, stop=True)
                xhp = pxh.tile([128, nch, n], f32, tag="pxh")
                for c in range(nch):
                    # xh: feat + ones-bias + o-feedback rows in one pass
                    m = (4 + c) * 128
                    nc.tensor.matmul(
                        xhp[:, c, :], mm_cast(kx_sb[:, m:m + 128]),
                        mm_cast(x), start=True, stop=True)

                # -- activation chain
                z_sb = zpool.tile([128, nch, n], cdt, tag="z")
                nc.scalar.activation(z_sb, zp, Act.Sigmoid)

                # rhh = (hh + rb_h) * r ; xh += rhh ; cand = tanh(xh)
                rh_sb = rhpool.tile([128, nch, n], cdt, tag="rh")
                for c in range(nch):
                    nc.vector.scalar_tensor_tensor(
                        rh_sb[:, c, :], hhp[:, c, :], rb_sb[:, c:c + 1],
                        r_sb[:, c, :], Alu.add, Alu.mult)
                nc.vector.tensor_tensor(xhp, xhp, rh_sb, Alu.add)
                cand = cpool.tile([128, nch, n], cdt, tag="cand")
                nc.scalar.activation(cand, xhp, Act.Tanh)

                # h_new = A - (z-1)*cand  with A = z*h (off the tanh chain)
                a_sb = apool.tile([128, nch, n], cdt, tag="a")
                nc.vector.tensor_tensor(a_sb, z_sb, h_prev[:, :, bs], Alu.mult)
                b_sb = bpool.tile([128, nch, n], cdt, tag="b")
                nc.vector.scalar_tensor_tensor(
                    b_sb, z_sb, -1.0, cand, Alu.add, Alu.mult)
                nc.vector.tensor_tensor(h_new[:, :, bs], a_sb, b_sb, Alu.subtract)

            h_hist.pop(t - 2, None)
            xs.pop((t - 1, 0), None)
            xs.pop((t - 1, 1), None)

        # final step's outputs
        for j in range(nt):
            op = emit_o_mm(t_steps - 1, j, h_hist[t_steps - 1])
            emit_o_copy(t_steps - 1, j, op)

    nc.compile()
    return nc


_NC_CACHE: dict = {}


def _get_nc(t_steps=T, bl=BL, nt=2, compute_dt="bfloat16"):
    key = (t_steps, bl, nt, compute_dt)
    if key not in _NC_CACHE:
        _NC_CACHE[key] = build_nc(t_steps, bl, nt, compute_dt)
    return _NC_CACHE[key]


def make_in_maps(
    decoder_feature,
    init_state,
    decoder_init_input,
    kernel,
    recurrent_kernel,
    input_bias,
    recurrent_bias,
    dense_w,
    dense_b,
    compute_dt="bfloat16",
    t_steps=T,
    bl=BL,
    ncores=NCORES,
    fold=True,
):
    np_c = np.float32 if compute_dt == "float32" else mybir.dt.np(mybir.dt.bfloat16)
    f = np.asarray(decoder_feature, np.float32)
    h0 = np.asarray(init_state, np.float32)
    o0 = np.asarray(decoder_init_input, np.float32)
    kx = np.asarray(kernel, np.float32)
    wr = np.asarray(recurrent_kernel, np.float32)
    ib = np.asarray(input_bias, np.float32)
    rb = np.asarray(recurrent_bias, np.float32)
    dw = np.asarray(dense_w, np.float32)
    db = float(np.asarray(dense_b, np.float32).reshape(-1)[0])
    k0 = kx[0]

    # bias row of the x-side stationary matrix: ib+rb for the z/r gate
    # columns (their mh/mx sum), ib only for the h columns (hh is biased
    # separately with rb inside the r* term), plus db routed through the
    # o-row weight (x row OROW carries o_raw = o - db).
    bias_row = np.concatenate([(ib + rb)[: 2 * H], ib[2 * H:]]) + db * k0
    kxw = np.zeros((XROWS, G3), np.float32)
    kxw[0:F] = kx[1:]               # feature rows
    kxw[ONESROW] = bias_row         # ones row (rows 65..95 stay zero)
    if fold:
        kxw[OROW, 2 * H:] = k0[2 * H:]  # o row: h cols only (z/r folded)
    else:
        kxw[OROW] = k0

    # z/r columns of the recurrent weights with the o-feedback fold
    wr_folded = wr.copy()
    if fold:
        wr_folded[:, : 2 * H] += dw @ k0[None, : 2 * H]

    in_maps = []
    for i in range(ncores):
        s = slice(i * bl, (i + 1) * bl)
        in_maps.append({
            "featT": np.ascontiguousarray(
                f[s, :t_steps].transpose(1, 2, 0)).astype(np_c),           # [T,F,bl]
            "h0T": np.ascontiguousarray(
                h0[s].T.reshape(2, 128, bl).transpose(1, 0, 2)).astype(np_c),
            "o0": np.ascontiguousarray((o0[s] - db).T).astype(np_c),       # [1, bl]
            "kxw": kxw.astype(np_c),
            "wrw": np.ascontiguousarray(
                wr_folded.reshape(2, 128, G3).transpose(1, 0, 2)).astype(np_c),
            "wrz0": np.ascontiguousarray(
                wr[:, : 2 * H].reshape(2, 128, 2 * H).transpose(1, 0, 2)).astype(np_c),
            "kxo": np.ascontiguousarray(k0[None, :]).astype(np_c),
            "dww": np.ascontiguousarray(dw.reshape(2, 128).T).astype(np_c),
            "ident": np.eye(128, dtype=np.float32).astype(np_c),
            "rbh": np.ascontiguousarray(
                rb[2 * H:].reshape(2, 128).T).astype(np.float32),
        })
    return in_maps, db


def run(inputs: dict, compute_dt="bfloat16", nt=2, trace=False, trace_kwargs=None):
    t_steps = int(inputs.get("predict_seq_length", T))
    assert t_steps == T, f"kernel hardcodes T={T}, got {t_steps}"
    nc = _get_nc(T, BL, nt, compute_dt)
    in_maps, db = make_in_maps(
        inputs["decoder_feature"], inputs["init_state"],
        inputs["decoder_init_input"], inputs["kernel"],
        inputs["recurrent_kernel"], inputs["input_bias"],
        inputs["recurrent_bias"], inputs["dense_w"], inputs["dense_b"],
        compute_dt=compute_dt,
    )
    res = bass_utils.run_bass_kernel_spmd(
        nc, in_maps, core_ids=list(range(NCORES)), trace=trace,
        **(trace_kwargs or {}),
    )
    out = np.empty((B, T, 1), np.float32)
    for i in range(NCORES):
        out[i * BL:(i + 1) * BL, :, 0] = res.results[i]["outT"].T + db
    return out, res


def kernel(**inputs) -> np.ndarray:
    out, _ = run(inputs)
    return out

